# revision 54
# baseline (speedup 1.0000x reference)
"""Trainium2 Bass kernel for NeuroFusionGNN (2x SAGEConv + LN + GELU + residual).

Self-contained: takes full inputs, shards across 8 NeuronCores internally,
returns the full [20000, 256] float32 output.

Strategy (hardcoded for N=20000, D=256, E=320000, 8 cores):
- Nodes sharded by dst: core c owns rows [c*2500, (c+1)*2500), processed as
  20 windows of 128 dst nodes (last window re-covers rows 2372..2500 and
  stores only its last 68 rows).
- Host preprocessing sorts edges by dst and emits, per (core, window):
  int16 gather indices (src node ids, padded to 128-multiples with idx 0)
  and bf16 "A" matrices A[e, d] = 1/deg(dst_e) one-hot on the window-local
  dst column (padding rows are all-zero).
- Device: dma_gather pulls neighbor rows (bf16) into [128e, 256] tiles;
  PE computes meanT[feat, dst] = sum_t msg_t[:, half].T @ A_t (PSUM f32);
  A carries exact small-int edge counts in fp8, the 1/deg scale applied
  post-matmul (AFP8). Root term transposes bf16 x rows on PE and matmuls
  against W_r.T; LayerNorm via bn_stats/bn_aggr + Newton rsqrt on DVE;
  GELU on ACT; residual in f32. Between layers an AllGather replicates
  fp8 x1 to all cores for the layer-2 gathers.

Measured structure (repeat-differential ablations on HW): each layer's
time is additive gather-phase + compute-phase — the SWDGE gather drain
runs at the full ~360GB/s DMA bus rate and does NOT overlap the compute
phase, so total time scales with bytes moved. fp8 messages + fp8 A halve
those bytes (~6-8% win); call-structure knobs (GCHUNK/queues/ring size/
store deferral/pipeline depth) are all neutral-or-worse vs the tuned
defaults (GCHUNK=4 across 4 queues).
"""
import os
import numpy as np
import ml_dtypes

import concourse.bacc as bacc
import concourse.tile as tile
import concourse.mybir as mybir
from concourse import bass_utils

N = 20000
D = 256
NCORES = 8
SHARD = N // NCORES          # 2500
WPC = 20                     # windows per core
WIN = 128
LAST_STORE = SHARD - (WPC - 1) * WIN   # 68
LAST_BASE = SHARD - WIN                # 2372
LN_EPS = 1e-5
GELU_MODE = "gelu"           # "gelu" (ACT table) or "erf" (erf-based exact)

def _env(k, d):
    return int(os.environ.get(k, str(d)))

CFG = dict(
    GCHUNK=_env("GCHUNK", 4),        # edge-tiles per dma_gather call
    NSWQ=_env("NSWQ", 4),            # SWDGE queues to spread gathers over
    DMA_SCRATCH=_env("DMA_SCRATCH", 16384),  # SWDGE ring bytes/partition
    FP8=_env("FP8", 0),              # gather messages in fp8e4m3: wins in
                                     # interleaved A/B benches (-16us) but
                                     # whole-process runs measure SLOWER
                                     # (343/302 vs 278us) — keep bf16
    NEWTON=_env("NEWTON", 1),        # rsqrt Newton steps
    XTACT=_env("XTACT", 1),          # xtsb copy on ACT vs DVE
    AFP8=_env("AFP8", 1),            # A one-hot in fp8 (exact small-int edge
                                     # counts; 1/deg applied post-matmul).
                                     # Halves A DMA bytes: ~8% faster (the
                                     # DMA bus is the binding resource)
    GFUSE=_env("GFUSE", 1),          # fuse LN normalize into GELU
    SPLITCP=_env("SPLITCP", 0),      # per-half PSUM->SBUF copies
    QSTRIDE=_env("QSTRIDE", 1),      # queue = (QSTRIDE*w + qi) % NSWQ
    QMODE=_env("QMODE", 0),          # 1: all of window w's chunks on ONE
                                     # queue (w % NSWQ) so 4 windows drain
                                     # concurrently instead of lockstep
    SPKT=_env("SPKT", 0),            # dma_gather single_packet
    NCV=_env("NCV", 0),              # constant gather count: WEDGES device
    IOBUFS=_env("IOBUFS", 3),
    MIDBUFS=_env("MIDBUFS", 3),
    PSBUFS=_env("PSBUFS", 2),        # agg/xt psum depth (banks: 2*PSBUFS)
    PHBUFS=_env("PHBUFS", 2),        # hP psum depth (banks: PHBUFS)
    SMBUFS=_env("SMBUFS", 4),        # small LN-tile pool depth
    FIXEDCNT=_env("FIXEDCNT", 0),    # all-valid padded chunks + hoisted
                                     # value_loads (GPSIMD op reduction)
    SQRTLN=_env("SQRTLN", 0),        # rsqrt = DVE reciprocal + ACT Sqrt
                                     # instead of magic+Newton DVE chain
    SHAREDAG=_env("SHAREDAG", 0),    # AllGather output in Shared DRAM
    ACACHE=_env("ACACHE", 0),        # keep all A tiles in SBUF across layers
                                     # (layer 2 reuses layer 1's loads)
    GWIN=_env("GWIN", 1),            # windows per gather group (one shared
                                     # msg tile + call set; needs FIXEDCNT)
    PADV=_env("PADV", 0),            # FIXEDCNT pad index: 0 = row 0 (hotspot),
                                     # 1 = spread across distinct rows
    HOIST=_env("HOIST", 1),          # FIXEDCNT: reuse hoisted value_loads
    AGC=_env("AGC", 1),              # AllGather chunks; >1 interleaves chunk
                                     # AGs into layer 0 (hides AG latency)
    RESBF16=_env("RESBF16", 0),      # keep x1 residual in bf16 only: gelu
                                     # emits bf16, residual add writes x1resb
                                     # directly (no f32 x1res, no ACT cast)
    STDELAY=_env("STDELAY", 0),      # defer stores K windows in program order:
                                     # HWDGE sem-waits happen at the SP
                                     # sequencer, so an unready store blocks
                                     # the next window's A/x loads (head-of-
                                     # line). Deferring restores prefetch.
    STQ=_env("STQ", 0),              # issue stores on the ACT HWDGE ring
                                     # instead of SP (separate FIFO)
)

f32 = mybir.dt.float32
bf16 = mybir.dt.bfloat16
f8 = mybir.dt.float8e4
i16 = mybir.dt.int16
i32 = mybir.dt.int32
Alu = mybir.AluOpType

_cache = {}


def _msgdt():
    return f8 if CFG["FP8"] else bf16


def _msgnp():
    return ml_dtypes.float8_e4m3 if CFG["FP8"] else ml_dtypes.bfloat16


def _chunks(T):
    """Per-group gather chunk boundaries. SWDGE desc-gen ~1us/call on the
    GPSIMD engine dominates, so fewer+larger calls win. Ring capacity needs
    DMA_SCRATCH//16 >= idxs per call."""
    nq = max(1, -(-T // CFG["GCHUNK"]))
    b = [round(j * T / nq) for j in range(nq + 1)]
    return [(b[j], b[j + 1]) for j in range(nq) if b[j + 1] > b[j]]


def _groups():
    """Partition windows into gather groups of GWIN windows each."""
    G = CFG["GWIN"]
    assert G == 1 or CFG["FIXEDCNT"], "GWIN>1 requires FIXEDCNT"
    return [list(range(g, min(g + G, WPC))) for g in range(0, WPC, G)]


def _ag_chunks():
    """AllGather chunk row ranges [(r0, r1), ...] aligned to window stores.
    Chunk k's agout block is contiguous (chunk-major layout): flat row
    NCORES*r0_k + c*(r1_k-r0_k) + (r-r0_k) holds core c's row r."""
    agc = CFG["AGC"]
    bnds = [round(k * WPC / agc) for k in range(agc + 1)]
    out = []
    for k in range(agc):
        r0 = bnds[k] * WIN
        r1 = bnds[k + 1] * WIN if bnds[k + 1] < WPC else SHARD
        out.append((min(r0, SHARD), min(r1, SHARD)))
    return out


def _ag_perm():
    """node id -> flat row in the chunk-major agout layout."""
    p = np.empty(N, dtype=np.int64)
    n = np.arange(N)
    c, r = n // SHARD, n % SHARD
    for (r0, r1) in _ag_chunks():
        m = (r >= r0) & (r < r1)
        p[n[m]] = NCORES * r0 + c[m] * (r1 - r0) + (r[m] - r0)
    return p


def _ncalls(Ts):
    return sum(len(_chunks(sum(Ts[w] for w in grp))) for grp in _groups())


def _preprocess(edge_index):
    src = np.asarray(edge_index[0], dtype=np.int64)
    dst = np.asarray(edge_index[1], dtype=np.int64)
    deg = np.bincount(dst, minlength=N)
    inv = (1.0 / np.maximum(deg, 1)).astype(np.float32)

    order = np.argsort(dst, kind="stable")
    ssrc = src[order]
    sdst = dst[order]

    # store-range boundaries for every (core, window)
    store_lo = np.array([c * SHARD + w * WIN for c in range(NCORES) for w in range(WPC)]
                        + [N], dtype=np.int64)
    starts = np.searchsorted(sdst, store_lo)

    # dedup (src, window): one gathered row per distinct src in a window;
    # its A row carries every dst column (values summed for parallel edges)
    uniq = {}
    ucnt = np.zeros((NCORES, WPC), dtype=np.int64)
    for c in range(NCORES):
        for w in range(WPC):
            k = c * WPC + w
            sl = slice(starts[k], starts[k + 1])
            s_u, r_of = np.unique(ssrc[sl], return_inverse=True)
            uniq[(c, w)] = (s_u, r_of)
            ucnt[c, w] = len(s_u)

    T = np.maximum(1, -(-ucnt.max(axis=0) // 128))         # tiles per window
    offs = np.concatenate([[0], np.cumsum(T)])             # tile offsets
    total_T = int(offs[-1])

    fixed = CFG["FIXEDCNT"]
    Tl = [int(t) for t in T]
    ncalls = _ncalls(Tl)
    perm = _ag_perm() if CFG["AGC"] > 1 else None
    idx_blobs, idx2_blobs, cnt_blobs, A_blobs, inv_blobs = [], [], [], [], []
    for c in range(NCORES):
        idxb = np.full((16, total_T * 8), -1, dtype=np.int16)
        Ab = np.zeros((128, total_T * 128), dtype=np.float32)
        invw = np.zeros((128, WPC), dtype=np.float32)
        cnts = np.zeros(ncalls, dtype=np.int32)
        for w in range(WPC):
            k = c * WPC + w
            sl = slice(starts[k], starts[k + 1])
            d = sdst[sl]
            s_u, r_of = uniq[(c, w)]
            n = len(s_u)
            base = c * SHARD + (w * WIN if w < WPC - 1 else LAST_BASE)
            col = d - base
            # AFP8: A carries exact small-int edge counts (fp8-exact); the
            # 1/deg scale is applied post-matmul from the invw blob.
            aval = np.ones(len(d), dtype=np.float32) if CFG["AFP8"] else inv[d]
            np.add.at(Ab, ((r_of & 127), (offs[w] + (r_of >> 7)) * 128 + col), aval)
            invw[:, w] = inv[base:base + WIN]
            r = np.arange(n)
            idxb[r % 16, offs[w] * 8 + r // 16] = s_u.astype(np.int16)
            if fixed:
                # pad every slot to a valid index: counts are the full
                # chunk size for every call, so one hoisted value_load per
                # distinct size serves all calls. A rows for pads are zero.
                p = np.arange(n, Tl[w] * 128)
                pv = (p % N).astype(np.int16) if CFG["PADV"] else 0
                idxb[p % 16, offs[w] * 8 + p // 16] = pv
        ci = 0
        for grp in _groups():
            Tg = sum(Tl[w] for w in grp)
            for (t0, t1) in _chunks(Tg):
                if fixed:
                    cnts[ci] = (t1 - t0) * 128
                else:
                    w = grp[0]
                    n = int(ucnt[c, w])
                    cnt = min(n, t1 * 128) - t0 * 128
                    if cnt <= 0:
                        # keep one valid dummy so the call is never empty
                        pos = t0 * 128
                        idxb[pos % 16, offs[w] * 8 + pos // 16] = 0
                        cnt = 1
                    cnts[ci] = cnt
                ci += 1
        assert ci == ncalls
        idx_blobs.append(np.tile(idxb, (8, 1)))
        if perm is not None:
            # layer 2 gathers from the chunk-major agout layout
            idxb2 = idxb.copy()
            vm = idxb2 >= 0
            idxb2[vm] = perm[idxb2[vm].astype(np.int64)].astype(np.int16)
            idx2_blobs.append(np.tile(idxb2, (8, 1)))
        A_blobs.append(Ab.astype(ml_dtypes.float8_e4m3 if CFG["AFP8"]
                                 else ml_dtypes.bfloat16))
        inv_blobs.append(invw)
        cnt_blobs.append(np.tile(cnts[None, :], (128, 1)))

    return ([int(x) for x in T], [int(x) for x in offs],
            idx_blobs, cnt_blobs, A_blobs, inv_blobs, idx2_blobs)


def _build(Ts, offs, use_b, use_g, use_be, single_core=False, compile=True,
           repeat=1, skip=frozenset(), full_repeat=1):
    """Build + compile the SPMD program. Returns nc.

    single_core=True replaces the AllGather with a local DRAM copy (for
    cost-model timing via TimelineSim, which is single-core only).
    repeat>1 wraps each layer in a device-side For_i loop (timing only)."""
    AFP8, FP8 = CFG["AFP8"], CFG["FP8"]
    GFUSE, SPLITCP, XTACT = CFG["GFUSE"], CFG["SPLITCP"], CFG["XTACT"]
    MSGDT = _msgdt()
    total_T = sum(Ts)
    nc = bacc.Bacc("TRN2", target_bir_lowering=False, debug=False,
                   num_devices=1 if single_core else NCORES,
                   num_swdge_queues=CFG["NSWQ"],
                   dynamic_dma_scratch_size=CFG["DMA_SCRATCH"])

    ncalls = _ncalls(Ts)
    xbf = nc.dram_tensor("xbf", [N, D], MSGDT, kind="ExternalInput").ap()
    xshbf = nc.dram_tensor("xshbf", [SHARD, D], bf16, kind="ExternalInput").ap()
    cntd = nc.dram_tensor("cnt", [128, ncalls], i32, kind="ExternalInput").ap()
    idxb = nc.dram_tensor("idxb", [128, total_T * 8], i16, kind="ExternalInput").ap()
    if CFG["AGC"] > 1:
        idxb2 = nc.dram_tensor("idxb2", [128, total_T * 8], i16,
                               kind="ExternalInput").ap()
    Ab = nc.dram_tensor("Ab", [128, total_T * 128], f8 if AFP8 else bf16,
                        kind="ExternalInput").ap()
    if AFP8:
        invw = nc.dram_tensor("invw", [128, WPC], f32, kind="ExternalInput").ap()
    ident = nc.dram_tensor("ident", [128, 128], bf16, kind="ExternalInput").ap()
    # weights: [128, 2, 256] bf16 (rhs of the linear matmuls)
    wls = [nc.dram_tensor(f"w{i}l", [128, 2, D], bf16, kind="ExternalInput").ap()
           for i in (1, 2)]
    wrs = [nc.dram_tensor(f"w{i}r", [128, 2, D], bf16, kind="ExternalInput").ap()
           for i in (1, 2)]
    bias_in = {}
    for i in (1, 2):
        if use_b:
            bias_in[f"b{i}"] = nc.dram_tensor(f"b{i}", [128, D], f32, kind="ExternalInput").ap()
        if use_g:
            bias_in[f"g{i}"] = nc.dram_tensor(f"g{i}", [128, D], f32, kind="ExternalInput").ap()
        if use_be:
            bias_in[f"be{i}"] = nc.dram_tensor(f"be{i}", [128, D], f32, kind="ExternalInput").ap()
    out = nc.dram_tensor("out", [SHARD, D], f32, kind="ExternalOutput").ap()

    MAGIC = 0x5F3759DF

    with tile.TileContext(nc) as tc:
        with (
            tc.tile_pool(name="cst", bufs=1) as cst,
            tc.tile_pool(name="io", bufs=CFG["IOBUFS"]) as io,
            tc.tile_pool(name="mid", bufs=CFG["MIDBUFS"]) as mid,
            tc.tile_pool(name="sm", bufs=CFG["SMBUFS"]) as sm,
            tc.tile_pool(name="ps", bufs=CFG["PSBUFS"], space="PSUM") as ps,
            tc.tile_pool(name="psx", bufs=CFG["PSBUFS"], space="PSUM") as psx,
            tc.tile_pool(name="psh", bufs=CFG["PHBUFS"], space="PSUM") as psh,
            tc.tile_pool(name="psr", bufs=CFG["PHBUFS"], space="PSUM") as psr,
            tc.tile_pool(name="dram", bufs=1, space="DRAM") as dram,
        ):
            idx_sb = cst.tile([128, total_T * 8], i16)
            nc.sync.dma_start(idx_sb[:], idxb)
            idx2_sb = idx_sb
            if CFG["AGC"] > 1:
                idx2_sb = cst.tile([128, total_T * 8], i16, name="idx2")
                nc.sync.dma_start(idx2_sb[:], idxb2)
            cnt_sb = cst.tile([128, ncalls], i32)
            nc.sync.dma_start(cnt_sb[:], cntd)
            id_sb = cst.tile([128, 128], bf16)
            nc.sync.dma_start(id_sb[:], ident)
            if AFP8:
                invw_sb = cst.tile([128, WPC], f32, name="invw")
                nc.sync.dma_start(invw_sb[:], invw)
            wl_sb, wr_sb, bias_sb = [], [], {}
            for i in (0, 1):
                wl = cst.tile([128, 2, D], bf16, name=f"wl{i}")
                nc.sync.dma_start(wl[:], wls[i])
                wl_sb.append(wl)
                wr = cst.tile([128, 2, D], bf16, name=f"wr{i}")
                nc.sync.dma_start(wr[:], wrs[i])
                wr_sb.append(wr)
                for key, use in ((f"b{i+1}", use_b), (f"g{i+1}", use_g), (f"be{i+1}", use_be)):
                    if use:
                        bt = cst.tile([128, D], f32, name=f"bias_{key}")
                        nc.sync.dma_start(bt[:], bias_in[key])
                        bias_sb[key] = bt

            # gather skips trailing-negative-padded rows, so matmuls read
            # stale SBUF there (times all-zero A columns). Zero every msg pool
            # slot once at full extent so stale bytes can never be NaN/Inf.
            maxTg = max(sum(Ts[w] for w in grp) for grp in _groups())
            for j in range(CFG["IOBUFS"]):
                mz = io.tile([128, maxTg, D], MSGDT, tag="msg", name=f"msgz_{j}")
                nc.vector.memset(mz[:], 0.0)

            eps_sb = None
            if CFG["SQRTLN"]:
                eps_sb = cst.tile([128, 1], f32, name="eps")
                nc.vector.memset(eps_sb[:], LN_EPS)

            A_all = None
            if CFG["ACACHE"]:
                # one tile per window: per-tile dep tracking, so layer-1
                # loads don't serialize against other windows' matmul reads
                A_all = [cst.tile([128, Ts[w] * 128], f8 if AFP8 else bf16,
                                  name=f"A_all_{w}") for w in range(WPC)]

            # FIXEDCNT: one value_load per distinct chunk size, reused by
            # every dma_gather call (hoisted off the per-window GPSIMD path)
            cv_cache = {}
            if CFG["FIXEDCNT"] and CFG["HOIST"] and "gather" not in skip:
                ci0 = 0
                for grp in _groups():
                    Tg = sum(Ts[w] for w in grp)
                    for (t0, t1) in _chunks(Tg):
                        v = (t1 - t0) * 128
                        if v not in cv_cache:
                            cv_cache[v] = nc.gpsimd.value_load(
                                cnt_sb[0:1, ci0:ci0 + 1])
                        ci0 += 1

            x1res = None
            if not CFG["RESBF16"]:
                x1res = cst.tile([128, WPC, D], f32)
            x1resb = cst.tile([128, WPC, D], bf16)   # bf16 copy (agin + xT src)
            if FP8:
                x1res8 = cst.tile([128, WPC, D], f8)  # fp8 copy for layer-2 gather
            x1f = dram.tile([WIN, D], bf16)  # rows LAST_BASE..SHARD only
            agin = dram.tile([SHARD, D], MSGDT)
            # chunk-major layout: each AG chunk's output block is contiguous
            # (the BIR verifier rejects strided collective outputs)
            agout = dram.tile([N, D], MSGDT,
                              addr_space="Shared" if CFG["SHAREDAG"] else "Local")

            def emit_ag(r0, r1):
                a0 = NCORES * r0
                if single_core:
                    nc.sync.dma_start(agout[a0:a0 + (r1 - r0), :], agin[r0:r1, :])
                else:
                    nc.gpsimd.collective_compute(
                        "AllGather", Alu.bypass,
                        replica_groups=[list(range(NCORES))],
                        ins=[agin[r0:r1, :].opt()],
                        outs=[agout[a0:NCORES * r1, :].opt()],
                    )

            # window w's agin store covers rows [w*128, (w+1)*128) (w<19) or
            # [2432, 2500) (w=19): chunk k fires once its windows all stored
            agc = CFG["AGC"]
            ag_after = {}
            if agc > 1:
                bnds = [round(k * WPC / agc) for k in range(agc + 1)]
                for k, (r0, r1) in enumerate(_ag_chunks()):
                    ag_after[bnds[k + 1] - 1] = (r0, r1)

            def emit_layer(layer, gsrc):
                ci = [0]
                pend = []
                for grp in _groups():
                    Tg = sum(Ts[w] for w in grp)
                    offg = offs[grp[0]]
                    msgg = io.tile([128, Tg, D], MSGDT, tag="msg",
                                   name=f"msg_{layer}_{grp[0]}")
                    if "gather" in skip:
                        nc.vector.memset(msgg[:, 0, 0:8], 0.0)
                    if "gather" not in skip:
                        # Split the group's tiles into chunks <= GCHUNK tiles
                        # (ring cap: DMA_SCRATCH//16 idxs/call). HW prefers
                        # more smaller calls spread over queues: the drain is
                        # random-read latency-bound and queue concurrency
                        # hides it.
                        for qi, (t0, t1) in enumerate(_chunks(Tg)):
                            if CFG["FIXEDCNT"] and CFG["HOIST"]:
                                cv = cv_cache[(t1 - t0) * 128]
                            elif CFG["NCV"]:
                                cv = (t1 - t0) * 128
                            else:
                                cv = nc.gpsimd.value_load(cnt_sb[0:1, ci[0]:ci[0] + 1])
                            isb = idx_sb if layer == 0 else idx2_sb
                            if CFG["QMODE"]:
                                qn = grp[0] % CFG["NSWQ"]
                            else:
                                qn = (CFG["QSTRIDE"] * grp[0] + qi) % CFG["NSWQ"]
                            nc.gpsimd.dma_gather(msgg[:, t0:t1, :], gsrc,
                                                 isb[:, (offg + t0) * 8:(offg + t1) * 8],
                                                 (t1 - t0) * 128, cv, D,
                                                 single_packet=bool(CFG["SPKT"]),
                                                 queue_num=qn)
                            ci[0] += 1
                    for w in grp:
                        while len(pend) > CFG["STDELAY"]:
                            pend.pop(0)()
                        emit_window(layer, w, msgg, offs[w] - offg, pend)
                        if layer == 0 and "store" not in skip and w in ag_after:
                            while pend:
                                pend.pop(0)()
                            emit_ag(*ag_after[w])
                for fn in pend:
                    fn()

            def emit_window(layer, w, msgg, woff, pend):
                    T = Ts[w]
                    off = offs[w]
                    last = w == WPC - 1
                    base = (w * WIN) if not last else LAST_BASE
                    st_lo = w * WIN
                    n_st = WIN if not last else LAST_STORE
                    pofs = 0 if not last else WIN - LAST_STORE
                    # (on-chip A generation via per-tile TensorScalarPtr was
                    # tried and measured SLOWER on HW: 548 vs 341 us)
                    if CFG["ACACHE"]:
                        if layer == 0 and "aload" not in skip:
                            nc.sync.dma_start(A_all[w][:],
                                              Ab[:, off * 128:(off + T) * 128])
                        A_t = lambda t: A_all[w][:, t * 128:(t + 1) * 128]
                    else:
                        A_sb = io.tile([128, T * 128], f8 if AFP8 else bf16,
                                       tag="A", name=f"A_{layer}_{w}")
                        if "aload" in skip:
                            nc.vector.memset(A_sb[:, 0:8], 0.0)
                        if "aload" not in skip:
                            nc.sync.dma_start(A_sb[:], Ab[:, off * 128:(off + T) * 128])
                        A_t = lambda t: A_sb[:, t * 128:(t + 1) * 128]
                    # xres: residual source; xT_sb: transposed root operand,
                    # loaded via HWDGE DMA-transpose (bf16 [feat, dst] tiles)
                    if layer == 0:
                        x_sb = io.tile([128, D], bf16, tag="x", name=f"x_{layer}_{w}")
                        if "xload" in skip:
                            nc.vector.memset(x_sb[:, 0:8], 0.0)
                        else:
                            nc.sync.dma_start(x_sb[:], xshbf[base:base + WIN, :])
                        xres = x_sb[:]
                    elif not last:
                        xres = (x1resb if CFG["RESBF16"] else x1res)[:, w, :]
                    else:
                        x_sb = io.tile([128, D], bf16, tag="x", name=f"x_{layer}_{w}")
                        if "xload" in skip:
                            nc.vector.memset(x_sb[:, 0:8], 0.0)
                        else:
                            nc.sync.dma_start(x_sb[:], x1f[:])
                        xres = x_sb[:]
                    # bf16 source for the PE root transpose
                    if layer == 0 or last:
                        xmm = lambda a, b: x_sb[:, a:b]
                    else:
                        xmm = lambda a, b: x1resb[:, w, a:b]

                    # aggregation: meanT halves accumulate over edge tiles.
                    # SPLITCP: drain each half's PSUM as soon as its group
                    # closes, overlapping the other half's matmuls.
                    aggps = ps.tile([128, 2, 128], f32, tag="agg", name=f"agg_{layer}_{w}")
                    aggsb = mid.tile([128, 2, 128], bf16, tag="aggsb", name=f"aggsb_{layer}_{w}")
                    Teff = 1 if "agg" in skip else T
                    for hh in range(2):
                        for t in range(Teff):
                            nc.tensor.matmul(aggps[:, hh, :],
                                             msgg[:, woff + t, hh * 128:(hh + 1) * 128],
                                             A_t(t),
                                             start=(t == 0), stop=(t == Teff - 1))
                        if SPLITCP:
                            nc.scalar.activation(aggsb[:, hh, :], aggps[:, hh, :],
                                                 mybir.ActivationFunctionType.Copy)
                    if not SPLITCP:
                        nc.scalar.activation(aggsb[:], aggps[:],
                                             mybir.ActivationFunctionType.Copy)

                    # root transpose on PE (bf16 in/out; copy out via DVE —
                    # ACT activation-copy misreads bf16 PSUM)
                    xtps = psx.tile([128, 2, 128], bf16, tag="xt", name=f"xt_{layer}_{w}")
                    xtsb = mid.tile([128, 2, 128], bf16, tag="xtsb", name=f"xtsb_{layer}_{w}")
                    for hh in range(1 if "xt" in skip else 2):
                        nc.tensor.transpose(xtps[:, hh, :],
                                            xmm(hh * 128, (hh + 1) * 128), id_sb[:])
                        if SPLITCP:
                            if XTACT:
                                nc.scalar.activation(xtsb[:, hh, :], xtps[:, hh, :],
                                                     mybir.ActivationFunctionType.Copy)
                            else:
                                nc.vector.tensor_copy(xtsb[:, hh, :], xtps[:, hh, :])
                    if not SPLITCP:
                        if XTACT:
                            nc.scalar.activation(xtsb[:], xtps[:],
                                                 mybir.ActivationFunctionType.Copy)
                        else:
                            nc.vector.tensor_copy(xtsb[:], xtps[:])

                    # linear: h = meanT.T @ WlT + xT.T @ WrT (all operands
                    # bf16/fp8, f32 PSUM accumulate)
                    if AFP8:
                        # split groups: agg part gets the 1/deg post-scale,
                        # root part doesn't; fused combine lands h in SBUF
                        hA = psh.tile([128, D], f32, tag="hA", name=f"hA_{layer}_{w}")
                        hR = psr.tile([128, D], f32, tag="hR", name=f"hR_{layer}_{w}")
                        if "linear" in skip:
                            nc.tensor.matmul(hA[:], aggsb[:, 0, :], wl_sb[layer][:, 0, :],
                                             start=True, stop=True)
                            nc.tensor.matmul(hR[:], xtsb[:, 0, :], wr_sb[layer][:, 0, :],
                                             start=True, stop=True)
                        else:
                            nc.tensor.matmul(hA[:], aggsb[:, 0, :], wl_sb[layer][:, 0, :],
                                             start=True, stop=False)
                            nc.tensor.matmul(hA[:], aggsb[:, 1, :], wl_sb[layer][:, 1, :],
                                             start=False, stop=True)
                            nc.tensor.matmul(hR[:], xtsb[:, 0, :], wr_sb[layer][:, 0, :],
                                             start=True, stop=False)
                            nc.tensor.matmul(hR[:], xtsb[:, 1, :], wr_sb[layer][:, 1, :],
                                             start=False, stop=True)
                        h_sb = mid.tile([128, D], f32, tag="h", name=f"h_{layer}_{w}")
                        # two ops: DVE can't read two PSUM operands at once
                        nc.vector.tensor_scalar(h_sb[:], hA[:],
                                                invw_sb[:, w:w + 1], None, Alu.mult)
                        nc.vector.tensor_tensor(h_sb[:], h_sb[:], hR[:], Alu.add)
                        if use_b:
                            nc.vector.tensor_tensor(h_sb[:], h_sb[:],
                                                    bias_sb[f"b{layer+1}"][:], Alu.add)
                        h_val = h_sb
                    else:
                        hps = psh.tile([128, D], f32, tag="hP", name=f"hP_{layer}_{w}")
                        if "linear" in skip:
                            nc.tensor.matmul(hps[:], aggsb[:, 0, :], wl_sb[layer][:, 0, :],
                                             start=True, stop=True)
                        else:
                            nc.tensor.matmul(hps[:], aggsb[:, 0, :], wl_sb[layer][:, 0, :],
                                             start=True, stop=False)
                            nc.tensor.matmul(hps[:], aggsb[:, 1, :], wl_sb[layer][:, 1, :],
                                             start=False, stop=False)
                            nc.tensor.matmul(hps[:], xtsb[:, 0, :], wr_sb[layer][:, 0, :],
                                             start=False, stop=False)
                            nc.tensor.matmul(hps[:], xtsb[:, 1, :], wr_sb[layer][:, 1, :],
                                             start=False, stop=True)
                        if use_b:
                            h_sb = mid.tile([128, D], f32, tag="h", name=f"h_{layer}_{w}")
                            nc.vector.tensor_tensor(h_sb[:], hps[:],
                                                    bias_sb[f"b{layer+1}"][:], Alu.add)
                            h_val = h_sb
                        else:
                            h_val = hps

                    fuse = (GFUSE and "ln" not in skip and GELU_MODE == "gelu"
                            and not use_g and not use_be)
                    y = None
                    if not fuse:
                        y = mid.tile([128, D], f32, tag="y", name=f"y_{layer}_{w}")
                    if "ln" in skip:
                        nc.vector.tensor_scalar(y[:], h_val[:], 1.0, None, Alu.mult)
                    else:
                        # LayerNorm stats
                        st6 = sm.tile([128, 6], f32, tag="st6", name=f"st6_{layer}_{w}")
                        nc.vector.bn_stats(st6[:], h_val[:])
                        mv = sm.tile([128, 2], f32, tag="mv", name=f"mv_{layer}_{w}")
                        nc.vector.bn_aggr(mv[:], st6[:])

                        rs = sm.tile([128, 1], f32, tag="rs", name=f"rs_{layer}_{w}")
                        if CFG["SQRTLN"]:
                            # sd = sqrt(var + eps) on ACT; rs = 1/sd on DVE
                            sd = sm.tile([128, 1], f32, tag="sd", name=f"sd_{layer}_{w}")
                            nc.scalar.activation(sd[:], mv[:, 1:2],
                                                 mybir.ActivationFunctionType.Sqrt,
                                                 bias=eps_sb[:, 0:1])
                            nc.vector.reciprocal(rs[:], sd[:])
                        else:
                            # inv_std = rsqrt(var + eps): magic + Newton (DVE)
                            va = sm.tile([128, 1], f32, tag="va", name=f"va_{layer}_{w}")
                            nc.vector.tensor_scalar(va[:], mv[:, 1:2], LN_EPS, None, Alu.add)
                            xi = sm.tile([128, 1], i32, tag="xi", name=f"xi_{layer}_{w}")
                            nc.vector.tensor_scalar(xi[:], va[:].bitcast(i32), 1, None,
                                                    Alu.arith_shift_right)
                            nc.vector.tensor_scalar(xi[:], xi[:], MAGIC, -1,
                                                    Alu.subtract, Alu.mult)
                            nc.vector.tensor_copy(rs[:], xi[:].bitcast(f32))
                            tmp = sm.tile([128, 1], f32, tag="tmp", name=f"tmp_{layer}_{w}")
                            for _ in range(CFG["NEWTON"]):
                                nc.vector.tensor_tensor(tmp[:], rs[:], rs[:], Alu.mult)
                                nc.vector.tensor_tensor(tmp[:], tmp[:], va[:], Alu.mult)
                                nc.vector.tensor_scalar(tmp[:], tmp[:], -0.5, 1.5,
                                                        Alu.mult, Alu.add)
                                nc.vector.tensor_tensor(rs[:], rs[:], tmp[:], Alu.mult)

                        if fuse:
                            # nb = (-mu) * rs; gelu reads h directly with
                            # act(rs*h + nb) == gelu((h - mu) * rs)
                            nb = sm.tile([128, 1], f32, tag="nb", name=f"nb_{layer}_{w}")
                            nc.vector.tensor_scalar(nb[:], mv[:, 0:1], -1.0, rs[:],
                                                    Alu.mult, Alu.mult)
                        else:
                            nc.vector.tensor_scalar(y[:], h_val[:], mv[:, 0:1], rs[:],
                                                    Alu.subtract, Alu.mult)
                    if use_g:
                        nc.vector.tensor_tensor(y[:], y[:],
                                                bias_sb[f"g{layer+1}"][:], Alu.mult)
                    if use_be:
                        nc.vector.tensor_tensor(y[:], y[:],
                                                bias_sb[f"be{layer+1}"][:], Alu.add)

                    if layer == 0:
                        xn_ap = (x1resb if CFG["RESBF16"] else x1res)[:, w, :]
                    else:
                        xn = mid.tile([128, D], f32, tag="xn", name=f"xn_{layer}_{w}")
                        xn_ap = xn[:]
                    if GELU_MODE in ("gelu", "tanh"):
                        fn = (mybir.ActivationFunctionType.Gelu if GELU_MODE == "gelu"
                              else mybir.ActivationFunctionType.Tanh)
                        gl = mid.tile([128, D], bf16 if CFG["RESBF16"] else f32,
                                      tag="gl", name=f"gl_{layer}_{w}")
                        if fuse:
                            nc.scalar.activation(gl[:], h_val[:], fn,
                                                 bias=nb[:, 0:1], scale=rs[:, 0:1])
                        else:
                            nc.scalar.activation(gl[:], y[:], fn)
                        nc.vector.tensor_tensor(xn_ap, xres, gl[:], Alu.add)
                    else:
                        er = mid.tile([128, D], f32, tag="gl", name=f"gl_{layer}_{w}")
                        nc.scalar.activation(er[:], y[:],
                                             mybir.ActivationFunctionType.Erf,
                                             scale=float(1.0 / np.sqrt(2.0)))
                        # z = (er + 1) * y ; xn = 0.5*z + x
                        nc.vector.scalar_tensor_tensor(er[:], er[:], 1.0, y[:],
                                                       Alu.add, Alu.mult)
                        nc.vector.scalar_tensor_tensor(xn_ap, er[:], 0.5, xres,
                                                       Alu.mult, Alu.add)

                    if layer == 0:
                        # bf16 cast on ACT, then HWDGE store — keeps the cast
                        # off Q7 (SWDGE cast-DMA contends with gather descgen);
                        # x1resb also feeds the layer-2 root transposes.
                        # RESBF16: the residual add wrote x1resb directly.
                        if not CFG["RESBF16"]:
                            nc.scalar.activation(x1resb[:, w, :], x1res[:, w, :],
                                                 mybir.ActivationFunctionType.Copy)
                        if FP8:
                            nc.vector.tensor_copy(x1res8[:, w, :], x1resb[:, w, :])
                            ag_src = x1res8
                        else:
                            ag_src = x1resb
                    st_eng = nc.scalar if CFG["STQ"] else nc.sync
                    if "store" in skip:
                        pass
                    elif layer == 0:
                        asrc = ag_src

                        def st0(asrc=asrc, w=w, st_lo=st_lo, n_st=n_st, pofs=pofs,
                                last=last):
                            st_eng.dma_start(agin[st_lo:st_lo + n_st, :],
                                             asrc[pofs:, w, :])
                            if w == WPC - 2:
                                # rows LAST_BASE..(WPC-1)*WIN: slot w's tail
                                st_eng.dma_start(
                                    x1f[0:WIN - LAST_STORE, :],
                                    x1resb[WIN - (WIN - LAST_STORE):, w, :])
                            elif last:
                                st_eng.dma_start(x1f[WIN - LAST_STORE:, :],
                                                 x1resb[pofs:, w, :])
                        pend.append(st0)
                    else:
                        def st1(xn=xn, st_lo=st_lo, n_st=n_st, pofs=pofs):
                            st_eng.dma_start(out[st_lo:st_lo + n_st, :],
                                             xn[pofs:, :])
                        pend.append(st1)

            if "l1" in skip:
                # timing layer 2 alone: feed it junk-but-finite layer-1 state
                if x1res is not None:
                    nc.vector.memset(x1res[:], 0.0)
                nc.vector.memset(x1resb[:], 0.0)
            layers = (0,) if "l2" in skip else ((1,) if "l1" in skip else (0, 1))
            for rep in range(full_repeat):
                for layer in layers:
                    gsrc = xbf if layer == 0 else agout[:]
                    if repeat > 1:
                        with tc.For_i(0, repeat, 1):
                            emit_layer(layer, gsrc)
                    else:
                        emit_layer(layer, gsrc)

                    if layer == 0 and agc == 1:
                        emit_ag(0, SHARD)

    if compile:
        nc.compile()
    return nc


def _make_in_maps(inputs, pre):
    Ts, offs, idx_blobs, cnt_blobs, A_blobs, inv_blobs, idx2_blobs = pre
    x = np.asarray(inputs["x"], dtype=np.float32)
    xbf = x.astype(ml_dtypes.bfloat16)
    xmsg = x.astype(_msgnp())

    def wdev(W, dtype):
        # rhs[k, j] = W[j, k]; layout [128 part=k%?, 2 khalf, 256 j]
        WT = np.ascontiguousarray(np.asarray(W, dtype=np.float32).T)  # [k, j]
        return WT.reshape(2, 128, D).transpose(1, 0, 2).astype(dtype).copy()

    b1 = np.asarray(inputs["b1l"], dtype=np.float32)
    b2 = np.asarray(inputs["b2l"], dtype=np.float32)
    g1 = np.asarray(inputs["g1"], dtype=np.float32)
    g2 = np.asarray(inputs["g2"], dtype=np.float32)
    be1 = np.asarray(inputs["be1"], dtype=np.float32)
    be2 = np.asarray(inputs["be2"], dtype=np.float32)
    use_b = not (np.all(b1 == 0) and np.all(b2 == 0))
    use_g = not (np.all(g1 == 1) and np.all(g2 == 1))
    use_be = not (np.all(be1 == 0) and np.all(be2 == 0))

    common = {
        "xbf": xmsg,
        "ident": np.eye(128, dtype=ml_dtypes.bfloat16),
        "w1l": wdev(inputs["W1l"], ml_dtypes.bfloat16),
        "w2l": wdev(inputs["W2l"], ml_dtypes.bfloat16),
        "w1r": wdev(inputs["W1r"], ml_dtypes.bfloat16),
        "w2r": wdev(inputs["W2r"], ml_dtypes.bfloat16),
    }
    if use_b:
        common["b1"] = np.tile(b1[None, :], (128, 1))
        common["b2"] = np.tile(b2[None, :], (128, 1))
    if use_g:
        common["g1"] = np.tile(g1[None, :], (128, 1))
        common["g2"] = np.tile(g2[None, :], (128, 1))
    if use_be:
        common["be1"] = np.tile(be1[None, :], (128, 1))
        common["be2"] = np.tile(be2[None, :], (128, 1))

    in_maps = []
    for c in range(NCORES):
        m = dict(common)
        m["xshbf"] = np.ascontiguousarray(xbf[c * SHARD:(c + 1) * SHARD, :])
        m["idxb"] = idx_blobs[c]
        if CFG["AGC"] > 1:
            m["idxb2"] = idx2_blobs[c]
        m["Ab"] = A_blobs[c]
        if CFG["AFP8"]:
            m["invw"] = inv_blobs[c]
        m["cnt"] = cnt_blobs[c]
        in_maps.append(m)
    return in_maps, (use_b, use_g, use_be)


def _prepare(inputs):
    edge_index = np.asarray(inputs["edge_index"])
    key = (hash(edge_index.tobytes()), tuple(sorted(CFG.items())))
    if key in _cache:
        return _cache[key]

    pre = _preprocess(edge_index)
    in_maps, (use_b, use_g, use_be) = _make_in_maps(inputs, pre)
    nc = _build(pre[0], pre[1], use_b, use_g, use_be)

    _cache[key] = (nc, in_maps)
    return nc, in_maps


def _assemble(res):
    return np.concatenate([np.asarray(res.results[c]["out"], dtype=np.float32)
                           for c in range(NCORES)], axis=0)


def kernel(**inputs):
    nc, in_maps = _prepare(inputs)
    res = bass_utils.run_bass_kernel_spmd(nc, in_maps, core_ids=list(range(NCORES)))
    return _assemble(res)


def run_traced(**inputs):
    """Returns (output, exec_time_ns or None). For test harness use."""
    nc, in_maps = _prepare(inputs)
    try:
        res = bass_utils.run_bass_kernel_spmd(
            nc, in_maps, core_ids=list(range(NCORES)), trace=True)
        return _assemble(res), res.exec_time_ns
    except Exception as e:  # trace/profile infra can fail independently of the run
        print(f"traced run failed ({e}); falling back to untraced")
        res = bass_utils.run_bass_kernel_spmd(nc, in_maps, core_ids=list(range(NCORES)))
        return _assemble(res), None
